# revision 19
# baseline (speedup 1.0000x reference)
"""MoE (top-2 of 8 experts, SwiGLU FFN) on 8 Trainium2 NeuronCores.

Strategy: expert-parallel with segment-based load balancing. The
gate/top-k routing is computed on host (bit-exact with the reference:
jax on CPU). Token-expert pairs are packed into 8 cores x S segments
(S=1 when every expert fits ceil(total_tiles/8) tiles, the common case;
S=2 otherwise, with the slot-1 weights reloaded into the slot-0 SBUF
tiles mid-run, hidden by per-h-tile WAR dependencies). Each segment
runs a dense SwiGLU FFN (bf16 matmuls, fp32 accumulation) over its
gathered tokens and scales rows by the renormalized top-k weight. The
host scatter-adds the per-segment outputs into the full [B,S,D] output.

Weights are host-packed h-major (w13 [24, 128, 2048], w2 [24, 128,
1024]) so each h-tile is ONE contiguous DMA: the PE can run complete
k-accumulations ~2us after launch instead of stalling ~25us on a
k-major w13 stream (k-accumulation needs all 8 k-tiles = the whole
12.6MB before the first PSUM group can retire).

Problem dims (hardcoded): B=4, S=2048, D=1024, E=8, TOP_K=2, H=3072.
"""

import sys
import types

if "/opt/trn_rl_repo" not in sys.path:
    sys.path.insert(0, "/opt/trn_rl_repo")

import numpy as np
import ml_dtypes


def _ensure_axon_hooks_shim():
    """bass_utils imports antenv.axon_hooks when BASS_TRACE is set; this
    image's antenv lacks it. Provide a no-op shim so tracing degrades
    gracefully instead of crashing (a real hook may overwrite it)."""
    try:
        import antenv.axon_hooks  # noqa: F401
        return
    except ImportError:
        pass
    try:
        import antenv
    except ImportError:
        return
    mod = types.ModuleType("antenv.axon_hooks")
    mod._hook = None
    mod.set_axon_ntff_profile_hook = lambda h: setattr(mod, "_hook", h)
    mod.get_axon_ntff_profile_hook = lambda: mod._hook
    sys.modules["antenv.axon_hooks"] = mod
    antenv.axon_hooks = mod


_ensure_axon_hooks_shim()

B, S, D = 4, 2048, 1024
E = 8
TOP_K = 2
H = 3 * D
T = B * S
KD = D // 128   # 8 k-tiles over D
NH = H // 128   # 24 h-tiles over H
ND = D // 512   # 2 512-wide output column tiles

BF16 = ml_dtypes.bfloat16

_nc_cache: dict = {}


def _chunks_for(c):
    """Token chunks: multiples of 128, PSUM free dim <= 512, and >= 384
    wherever possible so LDWEIGHTS stays hidden under the matmul
    stream. Prefers 512s; any small remainder chunk goes last."""
    tiles = c // 128
    for extra in ([], [256], [128], [256, 128]):
        t = tiles - sum(x // 128 for x in extra)
        if t < 0:
            continue
        for b in range(t // 4, -1, -1):
            if (t - 4 * b) % 3 == 0:
                out = [512] * b + [384] * ((t - 4 * b) // 3) + extra
                assert sum(out) == c, (c, out)
                return out
    return [128] * tiles


def build_ffn(caps: tuple):
    """Bass program for one core: len(caps) dense SwiGLU FFN segments.

    Segment s covers caps[s] tokens with weight slot s (each slot may
    hold a different expert's weights; slots > 0 reuse slot 0's SBUF).

    Inputs (host-prepacked, per core), for each slot s:
      xgk{s} [128, KD*caps[s]] bf16 : xgk[p, k*cap+c] = x[token c, k*128+p]
      w13{s} [NH, 128, KD*256] bf16 : h-major; [ht, p, k*256+j] =
          w1[k*128+p, ht*128+j] for j<128 else w3[k*128+p, ht*128+j-128]
      w2{s}  [NH, 128, D] bf16 : [ht, p, :] = w2[ht*128+p, :]
    plus
      wts [128, sum(caps)/128] f32 : weight of token n*128+p at [p, n]
    Output:
      yg [sum(caps), D] bf16 : wts * (silu(xg@w1) * (xg@w3)) @ w2
    """
    import concourse.bacc as bacc
    import concourse.tile as tile
    import concourse.mybir as mybir

    fp32 = mybir.dt.float32
    bf16 = mybir.dt.bfloat16

    assert all(c % 128 == 0 for c in caps)
    C = sum(caps)

    nc = bacc.Bacc("TRN2", target_bir_lowering=False, debug=False, num_devices=8)

    xgk, w13, w2 = [], [], []
    for s in range(len(caps)):
        xgk.append(nc.dram_tensor(f"xgk{s}", [128, KD * caps[s]], bf16,
                                  kind="ExternalInput"))
        w13.append(nc.dram_tensor(f"w13{s}", [NH, 128, KD * 256], bf16,
                                  kind="ExternalInput"))
        w2.append(nc.dram_tensor(f"w2{s}", [NH, 128, D], bf16,
                                 kind="ExternalInput"))
    wts = nc.dram_tensor("wts", [128, C // 128], fp32, kind="ExternalInput")
    yg = nc.dram_tensor("yg", [C, D], bf16, kind="ExternalOutput")

    with tile.TileContext(nc) as tc:
        with (
            tc.tile_pool(name="wres", bufs=1) as wres,
            tc.tile_pool(name="xgp", bufs=2) as xgp,
            tc.tile_pool(name="gp", bufs=1) as gp,
            tc.tile_pool(name="tmp", bufs=2) as tmp,
            tc.tile_pool(name="outp", bufs=2) as outp,
            # bufs is per-tag: ps1 x2 + ps3 x2 + pso x4 = 8 PSUM banks
            tc.tile_pool(name="psA", bufs=2, space="PSUM") as psA,
            tc.tile_pool(name="psB", bufs=4, space="PSUM") as psB,
        ):
            # DMA queue assignment (only gpsimd/sync/scalar may issue).
            # Early HBM bandwidth is scarce (~40-80 GB/s/core while all
            # 8 cores pull their weights), so EVERYTHING serializes
            # behind w13 on the sync queue except the two tensors
            # needed at t=0 (wts + chunk-0 tokens, on gpsimd): sync
            # carries w13 (h-major tiles -> first stage-A h-group can
            # retire ~10us in and the PE consumes tiles about as fast
            # as they arrive), then w2 (needed ~80us in), then later
            # token chunks and outputs in program order. scalar carries
            # only slot-1+ weight reloads (two-segment mode), whose
            # per-h-tile WAR on the slot-0 tiles clears progressively
            # during the previous segment's last chunk.
            wts_sb = wres.tile([128, C // 128], fp32, tag="wts")
            nc.gpsimd.dma_start(wts_sb[:], wts.ap())

            def load_xg_chunk(s, c0, NC, eng):
                # fixed-size buffer (chunks vary 256..512); data lands
                # in the first KD*NC columns
                xt = xgp.tile([128, KD * 512], bf16, tag="xg")
                eng.dma_start(
                    xt[:, :KD * NC].rearrange("p (k c) -> p k c", k=KD),
                    xgk[s].ap().rearrange("p (k c) -> p k c", k=KD)[:, :, c0:c0 + NC],
                )
                return xt

            def load_weights(s, eng13, eng2):
                w13_sb, w2_sb = [], []
                for ht in range(NH):
                    t1 = wres.tile([128, KD * 256], bf16, tag=f"w13_{ht}")
                    eng13.dma_start(t1[:], w13[s].ap()[ht])
                    w13_sb.append(t1)
                for ht in range(NH):
                    t2 = wres.tile([128, D], bf16, tag=f"w2_{ht}")
                    eng2.dma_start(t2[:], w2[s].ap()[ht])
                    w2_sb.append(t2)
                return w13_sb, w2_sb

            w13_sb, w2_sb = load_weights(0, nc.sync, nc.sync)
            first_chunks = _chunks_for(caps[0])
            xg_first = load_xg_chunk(0, 0, first_chunks[0], nc.gpsimd)

            gofs = 0  # global token-tile offset (for wts / yg indexing)
            for s, CS in enumerate(caps):
                chunks = _chunks_for(CS)
                if s > 0:
                    # reload weight slots; per-h-tile WAR keeps this off
                    # the critical path
                    w13_sb, w2_sb = load_weights(s, nc.scalar, nc.scalar)

                c0 = 0
                for ch, NC in enumerate(chunks):
                    NT = NC // 128
                    xg_t = (
                        xg_first if (s == 0 and ch == 0)
                        else load_xg_chunk(s, c0, NC, nc.sync)
                    )
                    xg_sb = [xg_t[:, k * NC:(k + 1) * NC] for k in range(KD)]

                    # stage A: g[h, tok] = silu(y1) * y3 for all 24 h-tiles
                    g_tiles = []
                    for ht in range(NH):
                        ps1 = psA.tile([128, 512], fp32, tag="ps1")
                        ps3 = psA.tile([128, 512], fp32, tag="ps3")
                        for k in range(KD):
                            nc.tensor.matmul(
                                ps1[:, :NC],
                                w13_sb[ht][:, k * 256:k * 256 + 128],
                                xg_sb[k],
                                start=(k == 0),
                                stop=(k == KD - 1),
                            )
                        for k in range(KD):
                            nc.tensor.matmul(
                                ps3[:, :NC],
                                w13_sb[ht][:, k * 256 + 128:k * 256 + 256],
                                xg_sb[k],
                                start=(k == 0),
                                stop=(k == KD - 1),
                            )
                        sig = tmp.tile([128, 512], fp32, tag="sig")
                        nc.scalar.activation(
                            sig[:, :NC], ps1[:, :NC],
                            mybir.ActivationFunctionType.Sigmoid,
                        )
                        sil = tmp.tile([128, 512], fp32, tag="sil")
                        nc.vector.tensor_mul(sil[:, :NC], sig[:, :NC], ps1[:, :NC])
                        gt = gp.tile([128, 512], bf16, tag=f"g_{ht}")
                        nc.vector.tensor_mul(gt[:, :NC], sil[:, :NC], ps3[:, :NC])
                        g_tiles.append(gt)

                    # stage B: yg[tok, d] = wts[tok] * (g.T @ w2).
                    # Outputs are staged per half-chunk and shipped as
                    # one batched DMA each (fewer descriptors + teardown
                    # semaphores than per-(tt,dh) stores).
                    for tt0 in range(0, NT, 2):
                        tts = range(tt0, min(tt0 + 2, NT))
                        ot = outp.tile([128, 2 * D], bf16, tag="ot")
                        for tt in tts:
                            gtile_idx = gofs + c0 // 128 + tt
                            for dh in range(ND):
                                pso = psB.tile([128, 512], fp32, tag="pso")
                                for ht in range(NH):
                                    nc.tensor.matmul(
                                        pso[:],
                                        g_tiles[ht][:, tt * 128:(tt + 1) * 128],
                                        w2_sb[ht][:, dh * 512:(dh + 1) * 512],
                                        start=(ht == 0),
                                        stop=(ht == NH - 1),
                                    )
                                col = (tt - tt0) * D + dh * 512
                                nc.vector.tensor_scalar_mul(
                                    ot[:, col:col + 512], pso[:],
                                    wts_sb[:, gtile_idx:gtile_idx + 1],
                                )
                        nt = len(tts)
                        r0 = gofs * 128 + c0 + tt0 * 128
                        nc.sync.dma_start(
                            yg.ap()[r0:r0 + nt * 128, :]
                            .rearrange("(tt p) d -> p tt d", p=128),
                            ot[:, :nt * D].rearrange("p (tt d) -> p tt d", tt=nt),
                        )
                    c0 += NC
                gofs += CS // 128

    nc.compile()
    return nc


def route_host(xf: np.ndarray, gate_w: np.ndarray):
    """Top-2 routing, bit-exact with the reference (jax on CPU)."""
    import jax
    import jax.numpy as jnp

    cpu = jax.devices("cpu")[0]
    with jax.default_device(cpu):
        xj = jax.device_put(xf, cpu)
        gj = jax.device_put(gate_w, cpu)
        probs = jax.nn.softmax(xj @ gj, axis=-1)
        vals, idx = jax.lax.top_k(probs, TOP_K)
        w = vals / jnp.sum(vals, axis=-1, keepdims=True)
    return np.asarray(idx), np.asarray(w)


def assign_pieces(counts, n_cores=8):
    """Partition expert loads onto n_cores cores of uniform capacity.

    Capacity is ceil(total_tiles / n_cores) tiles of 128 (retried +1 on
    greedy failure). If every expert fits one core's capacity (and
    there are <= n_cores experts), each core is ONE segment; otherwise
    each core is two segments (sizes capA >= capB) and big experts span
    multiple cores' segments. Returns (caps, assign): caps is the
    per-core segment-size tuple, assign a list of (expert, core, slot,
    tok_offset, n_tokens).
    """
    tiles = [(c + 127) // 128 for c in counts]
    tpc = max(2, (sum(tiles) + n_cores - 1) // n_cores)
    while True:
        if len(counts) <= n_cores and max(tiles) <= tpc:
            caps = (tpc * 128,)
            assign = [
                (e, core, 0, 0, counts[e])
                for core, e in enumerate(range(len(counts)))
            ]
            return caps, assign
        tA = (tpc + 1) // 2
        tB = tpc - tA
        availA, availB = n_cores, n_cores
        order = sorted(range(len(counts)), key=lambda e: -tiles[e])
        alloc = {e: [] for e in order}  # expert -> list of piece sizes
        ok = True
        for e in order:
            rem = tiles[e]
            while rem > 0:
                if availA and (rem >= tA or not availB):
                    alloc[e].append(tA)
                    availA -= 1
                    rem -= tA
                elif availB:
                    alloc[e].append(tB)
                    availB -= 1
                    rem -= tB
                else:
                    ok = False
                    break
            if not ok:
                break
        if ok:
            # leftover pieces go to the emptiest experts as padding
            for p in [tA] * availA + [tB] * availB:
                e = min(order, key=lambda e: sum(alloc[e]) - tiles[e])
                alloc[e].append(p)
            # each core = one tA piece (slot 0) + one tB piece (slot 1)
            coresA = list(range(n_cores))
            coresB = list(range(n_cores))
            assign = []
            for e in order:
                off = 0
                for p in sorted(alloc[e], reverse=True):
                    if p == tA and coresA:
                        core, slot = coresA.pop(0), 0
                    else:
                        core, slot = coresB.pop(0), 1
                    n = max(min(counts[e] - off, p * 128), 0)
                    assign.append((e, core, slot, off, n))
                    off += p * 128
            return (tA * 128, tB * 128), assign
        tpc += 1


def prepare_dispatch(x, gate_w):
    """Host routing + balanced segment assignment."""
    xf = np.ascontiguousarray(np.asarray(x).reshape(T, D), dtype=np.float32)
    gate_w = np.asarray(gate_w, dtype=np.float32)
    idx, w = route_host(xf, gate_w)
    tok_flat = np.repeat(np.arange(T), TOP_K)
    idx_flat = idx.ravel()
    w_flat = w.astype(np.float32).ravel()
    toks = []
    wts_list = []
    for e in range(E):
        sel = idx_flat == e
        toks.append(tok_flat[sel])
        wts_list.append(w_flat[sel])
    caps, assign = assign_pieces([len(t) for t in toks])
    return xf, toks, wts_list, caps, assign


def pack_xg(xf_bf, tok_slice, cap):
    """[128, KD*cap] bf16: xgk[p, k*cap+c] = x[token c, k*128+p]."""
    n = len(tok_slice)
    xgT = np.zeros((D, cap), dtype=BF16)
    if n:
        xgT[:, :n] = xf_bf[tok_slice].T
    return np.ascontiguousarray(
        xgT.reshape(KD, 128, cap).transpose(1, 0, 2).reshape(128, -1)
    )


def make_in_maps(xf, toks, wts_list, caps, assign, w1, w2, w3):
    xf_bf = xf.astype(BF16)
    # per-expert packed weights, shared (by reference) across slots
    w13p, w2p = [], []
    for e in range(E):
        w1e = np.asarray(w1[e], dtype=np.float32).astype(BF16)
        w3e = np.asarray(w3[e], dtype=np.float32).astype(BF16)
        # [NH, 128, KD*256]: [ht, p, k*256+j] = w1[k*128+p, ht*128+j] | w3
        w13 = np.concatenate(
            [
                w1e.reshape(KD, 128, NH, 128).transpose(2, 1, 0, 3),
                w3e.reshape(KD, 128, NH, 128).transpose(2, 1, 0, 3),
            ],
            axis=3,
        )  # [NH, 128, KD, 256]
        w13p.append(np.ascontiguousarray(w13.reshape(NH, 128, KD * 256)))
        w2p.append(
            np.ascontiguousarray(
                np.asarray(w2[e], dtype=np.float32).astype(BF16).reshape(NH, 128, D)
            )
        )

    C = sum(caps)
    base_of = np.concatenate([[0], np.cumsum(caps)]).astype(int)
    in_maps = [dict() for _ in range(E)]
    wts_full = [np.zeros(C, dtype=np.float32) for _ in range(E)]
    for e, core, slot, off, n in assign:
        m = in_maps[core]
        ts = toks[e][off:off + n]
        m[f"xgk{slot}"] = pack_xg(xf_bf, ts, caps[slot])
        m[f"w13{slot}"] = w13p[e]
        m[f"w2{slot}"] = w2p[e]
        wts_full[core][base_of[slot]:base_of[slot] + n] = wts_list[e][off:off + n]
    for core in range(E):
        # unassigned slots (pathological distributions only): zero
        # tokens + dummy weights
        for slot in range(len(caps)):
            if f"xgk{slot}" not in in_maps[core]:
                in_maps[core][f"xgk{slot}"] = np.zeros(
                    (128, KD * caps[slot]), dtype=BF16
                )
                in_maps[core][f"w13{slot}"] = w13p[0]
                in_maps[core][f"w2{slot}"] = w2p[0]
        in_maps[core]["wts"] = np.ascontiguousarray(
            wts_full[core].reshape(C // 128, 128).T
        )
    return in_maps


def combine_outputs(results, toks, caps, assign):
    base_of = np.concatenate([[0], np.cumsum(caps)]).astype(int)
    out = np.zeros((T, D), dtype=np.float32)
    for e, core, slot, off, n in assign:
        if n == 0:
            continue
        b = base_of[slot]
        yg = np.asarray(results[core]["yg"][b:b + n], dtype=np.float32)
        out[toks[e][off:off + n]] += yg  # token indices unique per segment
    return out.reshape(B, S, D)


def run(x, gate_w, w1, w2, w3, **spmd_kwargs):
    """Run the MoE. Returns (output, BassKernelResults)."""
    from concourse import bass_utils

    xf, toks, wts_list, caps, assign = prepare_dispatch(x, gate_w)
    if caps not in _nc_cache:
        _nc_cache[caps] = build_ffn(caps)
    nc = _nc_cache[caps]

    in_maps = make_in_maps(xf, toks, wts_list, caps, assign, w1, w2, w3)
    res = bass_utils.run_bass_kernel_spmd(
        nc, in_maps, core_ids=list(range(E)), **spmd_kwargs
    )
    out = combine_outputs(res.results, toks, caps, assign)
    out = out.astype(np.asarray(x).dtype, copy=False)
    return out, res


def kernel(x, gate_w, w1, w2, w3):
    out, _ = run(x, gate_w, w1, w2, w3)
    return out


# revision 22
# speedup vs baseline: 1.0177x; 1.0177x over previous
"""MoE (top-2 of 8 experts, SwiGLU FFN) on 8 Trainium2 NeuronCores.

Strategy: expert-parallel with segment-based load balancing. The
gate/top-k routing is computed on host (bit-exact with the reference:
jax on CPU). Token-expert pairs are packed into 8 cores x S segments
(S=1 when every expert fits ceil(total_tiles/8) tiles, the common case;
S=2 otherwise, with the slot-1 weights reloaded into the slot-0 SBUF
tiles mid-run, hidden by per-h-tile WAR dependencies). Each segment
runs a dense SwiGLU FFN (bf16 matmuls, fp32 accumulation) over its
gathered tokens and scales rows by the renormalized top-k weight. The
host scatter-adds the per-segment outputs into the full [B,S,D] output.

Weights are host-packed h-major (w13 [24, 128, 2048], w2 [24, 128,
1024]) so each h-tile is ONE contiguous DMA: the PE can run complete
k-accumulations ~2us after launch instead of stalling ~25us on a
k-major w13 stream (k-accumulation needs all 8 k-tiles = the whole
12.6MB before the first PSUM group can retire).

Problem dims (hardcoded): B=4, S=2048, D=1024, E=8, TOP_K=2, H=3072.
"""

import sys
import types

if "/opt/trn_rl_repo" not in sys.path:
    sys.path.insert(0, "/opt/trn_rl_repo")

import numpy as np
import ml_dtypes


def _ensure_axon_hooks_shim():
    """bass_utils imports antenv.axon_hooks when BASS_TRACE is set; this
    image's antenv lacks it. Provide a no-op shim so tracing degrades
    gracefully instead of crashing (a real hook may overwrite it)."""
    try:
        import antenv.axon_hooks  # noqa: F401
        return
    except ImportError:
        pass
    try:
        import antenv
    except ImportError:
        return
    mod = types.ModuleType("antenv.axon_hooks")
    mod._hook = None
    mod.set_axon_ntff_profile_hook = lambda h: setattr(mod, "_hook", h)
    mod.get_axon_ntff_profile_hook = lambda: mod._hook
    sys.modules["antenv.axon_hooks"] = mod
    antenv.axon_hooks = mod


_ensure_axon_hooks_shim()

B, S, D = 4, 2048, 1024
E = 8
TOP_K = 2
H = 3 * D
T = B * S
KD = D // 128   # 8 k-tiles over D
NH = H // 128   # 24 h-tiles over H
ND = D // 512   # 2 512-wide output column tiles

BF16 = ml_dtypes.bfloat16

_nc_cache: dict = {}


def _chunks_for(c):
    """Token chunks: multiples of 128, PSUM free dim <= 512, and >= 384
    wherever possible so LDWEIGHTS stays hidden under the matmul
    stream. Prefers 512s; any small remainder chunk goes last."""
    tiles = c // 128
    for extra in ([], [256], [128], [256, 128]):
        t = tiles - sum(x // 128 for x in extra)
        if t < 0:
            continue
        for b in range(t // 4, -1, -1):
            if (t - 4 * b) % 3 == 0:
                out = [512] * b + [384] * ((t - 4 * b) // 3) + extra
                assert sum(out) == c, (c, out)
                return out
    return [128] * tiles


def build_ffn(caps: tuple):
    """Bass program for one core: len(caps) dense SwiGLU FFN segments.

    Segment s covers caps[s] tokens with weight slot s (each slot may
    hold a different expert's weights; slots > 0 reuse slot 0's SBUF).

    Inputs (host-prepacked, per core), for each slot s:
      xgk{s} [128, KD*caps[s]] bf16 : xgk[p, k*cap+c] = x[token c, k*128+p]
      w13{s} [NH, 128, KD*256] bf16 : h-major; [ht, p, k*256+j] =
          w1[k*128+p, ht*128+j] for j<128 else w3[k*128+p, ht*128+j-128]
      w2{s}  [NH, 128, D] bf16 : [ht, p, :] = w2[ht*128+p, :]
    plus
      wts [128, sum(caps)/128] f32 : weight of token n*128+p at [p, n]
    Output:
      yg [sum(caps), D] bf16 : wts * (silu(xg@w1) * (xg@w3)) @ w2
    """
    import concourse.bacc as bacc
    import concourse.tile as tile
    import concourse.mybir as mybir

    fp32 = mybir.dt.float32
    bf16 = mybir.dt.bfloat16

    assert all(c % 128 == 0 for c in caps)
    C = sum(caps)

    nc = bacc.Bacc("TRN2", target_bir_lowering=False, debug=False, num_devices=8)

    xgk, w13, w2 = [], [], []
    for s in range(len(caps)):
        xgk.append(nc.dram_tensor(f"xgk{s}", [128, KD * caps[s]], bf16,
                                  kind="ExternalInput"))
        w13.append(nc.dram_tensor(f"w13{s}", [NH, 128, KD * 256], bf16,
                                  kind="ExternalInput"))
        w2.append(nc.dram_tensor(f"w2{s}", [NH, 128, D], bf16,
                                 kind="ExternalInput"))
    wts = nc.dram_tensor("wts", [128, C // 128], fp32, kind="ExternalInput")
    yg = nc.dram_tensor("yg", [C, D], bf16, kind="ExternalOutput")

    with tile.TileContext(nc) as tc:
        with (
            tc.tile_pool(name="wres", bufs=1) as wres,
            tc.tile_pool(name="xgp", bufs=2) as xgp,
            tc.tile_pool(name="gp", bufs=1) as gp,
            tc.tile_pool(name="tmp", bufs=2) as tmp,
            tc.tile_pool(name="outp", bufs=2) as outp,
            # bufs is per-tag: ps1 x2 + ps3 x2 + pso x4 = 8 PSUM banks
            tc.tile_pool(name="psA", bufs=2, space="PSUM") as psA,
            tc.tile_pool(name="psB", bufs=4, space="PSUM") as psB,
        ):
            # DMA queue assignment (only gpsimd/sync/scalar may issue).
            # Early HBM bandwidth is scarce (~40-80 GB/s/core while all
            # 8 cores pull their weights), so EVERYTHING serializes
            # behind w13 on the sync queue except the two tensors
            # needed at t=0 (wts + chunk-0 tokens, on gpsimd): sync
            # carries w13 (h-major tiles -> first stage-A h-group can
            # retire ~10us in and the PE consumes tiles about as fast
            # as they arrive), then w2 (needed ~80us in), then later
            # token chunks and outputs in program order. scalar carries
            # only slot-1+ weight reloads (two-segment mode), whose
            # per-h-tile WAR on the slot-0 tiles clears progressively
            # during the previous segment's last chunk.
            wts_sb = wres.tile([128, C // 128], fp32, tag="wts")
            nc.gpsimd.dma_start(wts_sb[:], wts.ap())

            def load_xg_chunk(s, c0, NC, eng):
                # fixed-size buffer (chunks vary 256..512); data lands
                # in the first KD*NC columns
                xt = xgp.tile([128, KD * 512], bf16, tag="xg")
                eng.dma_start(
                    xt[:, :KD * NC].rearrange("p (k c) -> p k c", k=KD),
                    xgk[s].ap().rearrange("p (k c) -> p k c", k=KD)[:, :, c0:c0 + NC],
                )
                return xt

            def load_weights(s, eng13, eng2):
                # ht=0 is split into two half-tiles (k=0..3 / k=4..7) so
                # the very first h-group's matmuls can start after only
                # 0.26MB has landed (readers of a tile wait on ALL its
                # writers, so halves must be separate tiles)
                w13_sb, w2_sb = [], []
                h0 = KD * 256 // 2
                t0a = wres.tile([128, h0], bf16, tag="w13_0a")
                eng13.dma_start(t0a[:], w13[s].ap()[0][:, :h0])
                t0b = wres.tile([128, h0], bf16, tag="w13_0b")
                eng13.dma_start(t0b[:], w13[s].ap()[0][:, h0:])
                w13_sb.append((t0a, t0b))
                for ht in range(1, NH):
                    t1 = wres.tile([128, KD * 256], bf16, tag=f"w13_{ht}")
                    eng13.dma_start(t1[:], w13[s].ap()[ht])
                    w13_sb.append(t1)
                for ht in range(NH):
                    t2 = wres.tile([128, D], bf16, tag=f"w2_{ht}")
                    eng2.dma_start(t2[:], w2[s].ap()[ht])
                    w2_sb.append(t2)
                return w13_sb, w2_sb

            def w13_slice(w13_sb, ht, k, second):
                # weight operand [128, 128] for (h-tile, k-tile); second
                # selects w3 columns
                col = k * 256 + (128 if second else 0)
                if ht == 0:
                    t = w13_sb[0][k // 4]
                    col -= (k // 4) * 4 * 256
                    return t[:, col:col + 128]
                return w13_sb[ht][:, col:col + 128]

            w13_sb, w2_sb = load_weights(0, nc.sync, nc.sync)
            first_chunks = _chunks_for(caps[0])
            # chunk-0 tokens as 8 separate k-slice tiles on the scalar
            # queue, in parallel with w13 on sync: the first matmul
            # needs only xg k=0 (128KB) + the first w13 half-tile
            NC0 = first_chunks[0]
            xgk0_3d = xgk[0].ap().rearrange("p (k c) -> p k c", k=KD)
            xg0_tiles = []
            for k in range(KD):
                xt = wres.tile([128, NC0], bf16, tag=f"xg0_{k}")
                nc.scalar.dma_start(xt[:], xgk0_3d[:, k, :NC0])
                xg0_tiles.append(xt)

            gofs = 0  # global token-tile offset (for wts / yg indexing)
            for s, CS in enumerate(caps):
                chunks = _chunks_for(CS)
                if s > 0:
                    # reload weight slots; per-h-tile WAR keeps this off
                    # the critical path
                    w13_sb, w2_sb = load_weights(s, nc.scalar, nc.scalar)

                c0 = 0
                for ch, NC in enumerate(chunks):
                    NT = NC // 128
                    if s == 0 and ch == 0:
                        xg_sb = [t[:] for t in xg0_tiles]
                    else:
                        xg_t = load_xg_chunk(s, c0, NC, nc.sync)
                        xg_sb = [xg_t[:, k * NC:(k + 1) * NC] for k in range(KD)]

                    # stage A: g[h, tok] = silu(y1) * y3 for all 24 h-tiles
                    g_tiles = []
                    for ht in range(NH):
                        ps1 = psA.tile([128, 512], fp32, tag="ps1")
                        ps3 = psA.tile([128, 512], fp32, tag="ps3")
                        if s == 0 and ch == 0 and ht == 0:
                            # k-major interleave: consume xg k-slices and
                            # w13 half-tiles in arrival order
                            for k in range(KD):
                                for second, ps in ((False, ps1), (True, ps3)):
                                    nc.tensor.matmul(
                                        ps[:, :NC],
                                        w13_slice(w13_sb, 0, k, second),
                                        xg_sb[k],
                                        start=(k == 0),
                                        stop=(k == KD - 1),
                                    )
                        else:
                            for second, ps in ((False, ps1), (True, ps3)):
                                for k in range(KD):
                                    nc.tensor.matmul(
                                        ps[:, :NC],
                                        w13_slice(w13_sb, ht, k, second),
                                        xg_sb[k],
                                        start=(k == 0),
                                        stop=(k == KD - 1),
                                    )
                        sig = tmp.tile([128, 512], fp32, tag="sig")
                        nc.scalar.activation(
                            sig[:, :NC], ps1[:, :NC],
                            mybir.ActivationFunctionType.Sigmoid,
                        )
                        sil = tmp.tile([128, 512], fp32, tag="sil")
                        nc.vector.tensor_mul(sil[:, :NC], sig[:, :NC], ps1[:, :NC])
                        gt = gp.tile([128, 512], bf16, tag=f"g_{ht}")
                        nc.vector.tensor_mul(gt[:, :NC], sil[:, :NC], ps3[:, :NC])
                        g_tiles.append(gt)

                    # stage B: yg[tok, d] = wts[tok] * (g.T @ w2).
                    # Outputs staged per token-tile and shipped as one
                    # contiguous [128, D] DMA each (fewer descriptors +
                    # teardown semaphores than per-(tt,dh) stores).
                    for tt in range(NT):
                        gtile_idx = gofs + c0 // 128 + tt
                        ot = outp.tile([128, D], bf16, tag="ot")
                        for dh in range(ND):
                            pso = psB.tile([128, 512], fp32, tag="pso")
                            for ht in range(NH):
                                nc.tensor.matmul(
                                    pso[:],
                                    g_tiles[ht][:, tt * 128:(tt + 1) * 128],
                                    w2_sb[ht][:, dh * 512:(dh + 1) * 512],
                                    start=(ht == 0),
                                    stop=(ht == NH - 1),
                                )
                            nc.vector.tensor_scalar_mul(
                                ot[:, dh * 512:(dh + 1) * 512], pso[:],
                                wts_sb[:, gtile_idx:gtile_idx + 1],
                            )
                        r0 = gofs * 128 + c0 + tt * 128
                        nc.sync.dma_start(yg.ap()[r0:r0 + 128, :], ot[:])
                    c0 += NC
                gofs += CS // 128

    nc.compile()
    return nc


def route_host(xf: np.ndarray, gate_w: np.ndarray):
    """Top-2 routing, bit-exact with the reference (jax on CPU)."""
    import jax
    import jax.numpy as jnp

    cpu = jax.devices("cpu")[0]
    with jax.default_device(cpu):
        xj = jax.device_put(xf, cpu)
        gj = jax.device_put(gate_w, cpu)
        probs = jax.nn.softmax(xj @ gj, axis=-1)
        vals, idx = jax.lax.top_k(probs, TOP_K)
        w = vals / jnp.sum(vals, axis=-1, keepdims=True)
    return np.asarray(idx), np.asarray(w)


def assign_pieces(counts, n_cores=8):
    """Partition expert loads onto n_cores cores of uniform capacity.

    Capacity is ceil(total_tiles / n_cores) tiles of 128 (retried +1 on
    greedy failure). If every expert fits one core's capacity (and
    there are <= n_cores experts), each core is ONE segment; otherwise
    each core is two segments (sizes capA >= capB) and big experts span
    multiple cores' segments. Returns (caps, assign): caps is the
    per-core segment-size tuple, assign a list of (expert, core, slot,
    tok_offset, n_tokens).
    """
    tiles = [(c + 127) // 128 for c in counts]
    tpc = max(2, (sum(tiles) + n_cores - 1) // n_cores)
    while True:
        if len(counts) <= n_cores and max(tiles) <= tpc:
            caps = (tpc * 128,)
            assign = [
                (e, core, 0, 0, counts[e])
                for core, e in enumerate(range(len(counts)))
            ]
            return caps, assign
        tA = (tpc + 1) // 2
        tB = tpc - tA
        availA, availB = n_cores, n_cores
        order = sorted(range(len(counts)), key=lambda e: -tiles[e])
        alloc = {e: [] for e in order}  # expert -> list of piece sizes
        ok = True
        for e in order:
            rem = tiles[e]
            while rem > 0:
                if availA and (rem >= tA or not availB):
                    alloc[e].append(tA)
                    availA -= 1
                    rem -= tA
                elif availB:
                    alloc[e].append(tB)
                    availB -= 1
                    rem -= tB
                else:
                    ok = False
                    break
            if not ok:
                break
        if ok:
            # leftover pieces go to the emptiest experts as padding
            for p in [tA] * availA + [tB] * availB:
                e = min(order, key=lambda e: sum(alloc[e]) - tiles[e])
                alloc[e].append(p)
            # each core = one tA piece (slot 0) + one tB piece (slot 1)
            coresA = list(range(n_cores))
            coresB = list(range(n_cores))
            assign = []
            for e in order:
                off = 0
                for p in sorted(alloc[e], reverse=True):
                    if p == tA and coresA:
                        core, slot = coresA.pop(0), 0
                    else:
                        core, slot = coresB.pop(0), 1
                    n = max(min(counts[e] - off, p * 128), 0)
                    assign.append((e, core, slot, off, n))
                    off += p * 128
            return (tA * 128, tB * 128), assign
        tpc += 1


def prepare_dispatch(x, gate_w):
    """Host routing + balanced segment assignment."""
    xf = np.ascontiguousarray(np.asarray(x).reshape(T, D), dtype=np.float32)
    gate_w = np.asarray(gate_w, dtype=np.float32)
    idx, w = route_host(xf, gate_w)
    tok_flat = np.repeat(np.arange(T), TOP_K)
    idx_flat = idx.ravel()
    w_flat = w.astype(np.float32).ravel()
    toks = []
    wts_list = []
    for e in range(E):
        sel = idx_flat == e
        toks.append(tok_flat[sel])
        wts_list.append(w_flat[sel])
    caps, assign = assign_pieces([len(t) for t in toks])
    return xf, toks, wts_list, caps, assign


def pack_xg(xf_bf, tok_slice, cap):
    """[128, KD*cap] bf16: xgk[p, k*cap+c] = x[token c, k*128+p]."""
    n = len(tok_slice)
    xgT = np.zeros((D, cap), dtype=BF16)
    if n:
        xgT[:, :n] = xf_bf[tok_slice].T
    return np.ascontiguousarray(
        xgT.reshape(KD, 128, cap).transpose(1, 0, 2).reshape(128, -1)
    )


def make_in_maps(xf, toks, wts_list, caps, assign, w1, w2, w3):
    xf_bf = xf.astype(BF16)
    # per-expert packed weights, shared (by reference) across slots
    w13p, w2p = [], []
    for e in range(E):
        w1e = np.asarray(w1[e], dtype=np.float32).astype(BF16)
        w3e = np.asarray(w3[e], dtype=np.float32).astype(BF16)
        # [NH, 128, KD*256]: [ht, p, k*256+j] = w1[k*128+p, ht*128+j] | w3
        w13 = np.concatenate(
            [
                w1e.reshape(KD, 128, NH, 128).transpose(2, 1, 0, 3),
                w3e.reshape(KD, 128, NH, 128).transpose(2, 1, 0, 3),
            ],
            axis=3,
        )  # [NH, 128, KD, 256]
        w13p.append(np.ascontiguousarray(w13.reshape(NH, 128, KD * 256)))
        w2p.append(
            np.ascontiguousarray(
                np.asarray(w2[e], dtype=np.float32).astype(BF16).reshape(NH, 128, D)
            )
        )

    C = sum(caps)
    base_of = np.concatenate([[0], np.cumsum(caps)]).astype(int)
    in_maps = [dict() for _ in range(E)]
    wts_full = [np.zeros(C, dtype=np.float32) for _ in range(E)]
    for e, core, slot, off, n in assign:
        m = in_maps[core]
        ts = toks[e][off:off + n]
        m[f"xgk{slot}"] = pack_xg(xf_bf, ts, caps[slot])
        m[f"w13{slot}"] = w13p[e]
        m[f"w2{slot}"] = w2p[e]
        wts_full[core][base_of[slot]:base_of[slot] + n] = wts_list[e][off:off + n]
    for core in range(E):
        # unassigned slots (pathological distributions only): zero
        # tokens + dummy weights
        for slot in range(len(caps)):
            if f"xgk{slot}" not in in_maps[core]:
                in_maps[core][f"xgk{slot}"] = np.zeros(
                    (128, KD * caps[slot]), dtype=BF16
                )
                in_maps[core][f"w13{slot}"] = w13p[0]
                in_maps[core][f"w2{slot}"] = w2p[0]
        in_maps[core]["wts"] = np.ascontiguousarray(
            wts_full[core].reshape(C // 128, 128).T
        )
    return in_maps


def combine_outputs(results, toks, caps, assign):
    base_of = np.concatenate([[0], np.cumsum(caps)]).astype(int)
    out = np.zeros((T, D), dtype=np.float32)
    for e, core, slot, off, n in assign:
        if n == 0:
            continue
        b = base_of[slot]
        yg = np.asarray(results[core]["yg"][b:b + n], dtype=np.float32)
        out[toks[e][off:off + n]] += yg  # token indices unique per segment
    return out.reshape(B, S, D)


def run(x, gate_w, w1, w2, w3, **spmd_kwargs):
    """Run the MoE. Returns (output, BassKernelResults)."""
    from concourse import bass_utils

    xf, toks, wts_list, caps, assign = prepare_dispatch(x, gate_w)
    if caps not in _nc_cache:
        _nc_cache[caps] = build_ffn(caps)
    nc = _nc_cache[caps]

    in_maps = make_in_maps(xf, toks, wts_list, caps, assign, w1, w2, w3)
    res = bass_utils.run_bass_kernel_spmd(
        nc, in_maps, core_ids=list(range(E)), **spmd_kwargs
    )
    out = combine_outputs(res.results, toks, caps, assign)
    out = out.astype(np.asarray(x).dtype, copy=False)
    return out, res


def kernel(x, gate_w, w1, w2, w3):
    out, _ = run(x, gate_w, w1, w2, w3)
    return out
